# revision 10
# baseline (speedup 1.0000x reference)
"""Block-causal attention block (RMSnorm + QKV + frame-causal attention + proj)
on 8 TRN2 NeuronCores.

Sharding: sequence-parallel over the 8 frames — core i owns the 1024 queries of
frame i and processes KV blocks for frames 0..i (uniform SPMD program: all 16
half-blocks are processed on every core; future frames are killed by a
per-core additive bias of -1e30 before the exp, so they contribute exp() = 0
to both the numerator and denominator of the softmax).

Layouts are channel-first throughout ([C, seq] with C on partitions), which
makes every contraction a natural PE matmul with no transposes:
  k^T [C, kv]   = Wk' @ xn           (lhsT = wkT chunk, rhs = xn)
  v   [kv, C]   = xn^T @ Wv'^T       (lhsT = xn chunk,  rhs = wvT)
  S^T [kv, q]   = K @ Q^T            (lhsT = k^T chunk, rhs = q^T)
  O^T [C, q]    = V^T @ P^T          (lhsT = v chunk,   rhs = p^T)
  den [1, q]    = ones^T @ P^T       (lhsT = ones,      rhs = p^T)

Host-side folds: gamma*sqrt(C) into wq/wk/wv; bv through wp into the output
bias (softmax rows sum to 1); no max-subtraction in the softmax (scores here
are O(1); exp is safe and matches jax.nn.softmax exactly up to rounding).

All matmuls run in float32r (FP22) — full-rate on TRN2 with ~1e-4 accuracy.
"""

import sys

import numpy as np

sys.path.insert(0, "/opt/trn_rl_repo")

import concourse.bacc as bacc
import concourse.bass as bass  # noqa: F401
import concourse.tile as tile
from concourse import mybir
from concourse.bass_utils import run_bass_kernel_spmd

C = 512
CC = C // 128          # 4 channel chunks
F = 8                  # frames
HW = 1024              # tokens per frame
SEQ = F * HW           # 8192
S = 512                # kv columns processed per step
NSTEP = SEQ // S       # 16
Q = 1024               # queries per core (one frame)
QH = Q // S            # 2 query halves
KVC = SEQ // 128       # 64 kv 128-chunks
SCALE = 1.0 / float(np.sqrt(C))
NEG = -1.0e30

F32 = mybir.dt.float32
F32R = mybir.dt.float32r
Act = mybir.ActivationFunctionType

_cached = {}


def _build():
    if "nc" in _cached:
        return _cached["nc"]

    nc = bacc.Bacc()
    xq_d = nc.dram_tensor("xq", [C, Q], F32, kind="ExternalInput")
    xkv_d = nc.dram_tensor("xkv", [C, SEQ], F32, kind="ExternalInput")
    kvb_d = nc.dram_tensor("kvb", [128, KVC], F32, kind="ExternalInput")
    wq_d = nc.dram_tensor("wqT", [C, C], F32, kind="ExternalInput")
    wk_d = nc.dram_tensor("wkT", [C, C], F32, kind="ExternalInput")
    wv_d = nc.dram_tensor("wvT", [C, C], F32, kind="ExternalInput")
    wp_d = nc.dram_tensor("wpT", [C, C], F32, kind="ExternalInput")
    bq_d = nc.dram_tensor("bq", [C, 1], F32, kind="ExternalInput")
    bk_d = nc.dram_tensor("bk", [C, 1], F32, kind="ExternalInput")
    bvp_d = nc.dram_tensor("bvp", [C, 1], F32, kind="ExternalInput")
    out_d = nc.dram_tensor("out", [C, Q], F32, kind="ExternalOutput")

    with tile.TileContext(nc) as tc:
        with (
            tc.tile_pool(name="const", bufs=1) as const,
            tc.tile_pool(name="persist", bufs=1) as persist,
            tc.tile_pool(name="xload", bufs=2) as xload,
            tc.tile_pool(name="norm", bufs=2) as norm,
            tc.tile_pool(name="kv", bufs=2) as kvpool,
            tc.tile_pool(name="ppool", bufs=2) as ppool,
            tc.tile_pool(name="dram", bufs=1, space="DRAM") as drampool,
            tc.tile_pool(name="psum_s", bufs=2, space="PSUM") as psum_s,
            tc.tile_pool(name="psum_o", bufs=2, space="PSUM") as psum_o,
            tc.tile_pool(name="psum_den", bufs=2, space="PSUM") as psum_den,
        ):
            # ---- constants / weights (wq and wp share one slot: wp is only
            # needed after the last use of wq) ----
            wq_sb = const.tile([128, CC, C], F32R, tag="wqp", name="wq_sb")
            wk_sb = const.tile([128, CC, C], F32R, tag="wk", name="wk_sb")
            wv_sb = const.tile([128, CC, C], F32R, tag="wv", name="wv_sb")
            for w_sb, w_d in ((wq_sb, wq_d), (wk_sb, wk_d), (wv_sb, wv_d)):
                for ci in range(CC):
                    nc.sync.dma_start(
                        out=w_sb[:, ci, :],
                        in_=w_d[ci * 128:(ci + 1) * 128, :].bitcast(F32R),
                    )
            bq_sb = const.tile([128, CC], F32, tag="bq", name="bq_sb")
            bk_sb = const.tile([128, CC], F32, tag="bk", name="bk_sb")
            bvp_sb = const.tile([128, CC], F32, tag="bvp", name="bvp_sb")
            for b_sb, b_d in ((bq_sb, bq_d), (bk_sb, bk_d), (bvp_sb, bvp_d)):
                for ci in range(CC):
                    nc.sync.dma_start(
                        out=b_sb[:, ci:ci + 1],
                        in_=b_d[ci * 128:(ci + 1) * 128, :],
                    )
            kvb_sb = const.tile([128, KVC], F32, tag="kvb", name="kvb_sb")
            nc.sync.dma_start(out=kvb_sb[:], in_=kvb_d[:])
            ones_f = const.tile([128, 1], F32, tag="ones_f", name="ones_f")
            nc.vector.memset(ones_f[:], 1.0)
            ones_sb = const.tile([128, 1], F32R, tag="ones", name="ones_sb")
            nc.vector.tensor_copy(ones_sb[:], ones_f[:])
            # PE warmup: ~7us of back-to-back matmuls so the HAM clock gate
            # opens (4/8 -> 8/8) before the real matmul stream begins
            warm_f = const.tile([128, S], F32, tag="warm_f", name="warm_f")
            nc.vector.memset(warm_f[:], 0.0)
            warm_r = const.tile([128, S], F32R, tag="warm_r", name="warm_r")
            nc.vector.tensor_copy(warm_r[:], warm_f[:])
            warm_ps = psum_s.tile([1, S], F32, tag="s", name="warm_ps")
            for wi in range(20):
                nc.tensor.matmul(
                    warm_ps[:], ones_sb[:], warm_r[:],
                    start=(wi == 0), stop=(wi == 19),
                )

            # ---- persistent q-side tiles ----
            qT_sb = persist.tile([128, CC, Q], F32R, tag="qT", name="qT_sb")
            o_sb = persist.tile([128, CC, Q], F32, tag="o", name="o_sb")
            rdb = persist.tile([128, Q], F32, tag="rdb", name="rdb")
            den_sb = persist.tile([1, Q], F32, tag="den_sb", name="den_sb")
            nc.vector.memset(den_sb[:], 0.0)

            # ---- stats prepass: per-column ln(sum x^2) for q halves (u=0,1)
            # and kv steps (u=2..17), parked in DRAM; the main loop
            # broadcast-DMAs each row back and applies exp(-0.5*ln) so the
            # scalar engine only ever needs Ln and Exp tables ----
            ln_dram = drampool.tile([18, S], F32, tag="ln_dram", name="ln_dram")

            def stats_step(u, x_dram, col0):
                ss_ps = psum_den.tile([1, S], F32, tag="small", name="ss_ps")
                for ci in range(CC):
                    xp = norm.tile([128, S], F32, tag="xp", name="xp")
                    nc.sync.dma_start(
                        out=xp[:],
                        in_=x_dram[ci * 128:(ci + 1) * 128, col0:col0 + S],
                    )
                    xsq = norm.tile([128, S], F32R, tag="xsq", name="xsq")
                    nc.vector.tensor_mul(xsq[:], xp[:], xp[:])
                    nc.tensor.matmul(
                        ss_ps[:], ones_sb[:], xsq[:],
                        start=(ci == 0), stop=(ci == CC - 1),
                    )
                ln_t = norm.tile([1, S], F32, tag="ln_t", name="ln_t")
                nc.scalar.activation(ln_t[:], ss_ps[:], Act.Ln)
                nc.sync.dma_start(out=ln_dram[u:u + 1, :], in_=ln_t[:])

            for qh in range(QH):
                stats_step(qh, xq_d, qh * S)
            for t in range(NSTEP):
                stats_step(2 + t, xkv_d, t * S)

            def load_xn(u, x_dram, col0):
                """Reload x slab and normalize columns: xn = x * exp(-ln/2)."""
                xt = xload.tile([128, CC, S], F32, tag="xt", name="xt")
                for ci in range(CC):
                    nc.sync.dma_start(
                        out=xt[:, ci, :],
                        in_=x_dram[ci * 128:(ci + 1) * 128, col0:col0 + S],
                    )
                lnb = norm.tile([128, S], F32, tag="lnb", name="lnb")
                nc.sync.dma_start(
                    out=lnb[:], in_=ln_dram[u:u + 1, :].to_broadcast([128, S])
                )
                rnb = norm.tile([128, S], F32, tag="rnb", name="rnb")
                nc.scalar.activation(rnb[:], lnb[:], Act.Exp, scale=-0.5)
                xn = norm.tile([128, CC, S], F32R, tag="xn", name="xn")
                for ci in range(CC):
                    nc.vector.tensor_mul(xn[:, ci, :], xt[:, ci, :], rnb[:])
                return xn

            # ---- Q path (once) ----
            for qh in range(QH):
                xn = load_xn(qh, xq_d, qh * S)
                for co in range(CC):
                    q_ps = psum_o.tile([128, S], F32, tag="proj", name="q_ps")
                    for ci in range(CC):
                        nc.tensor.matmul(
                            q_ps[:],
                            wq_sb[:, ci, co * 128:(co + 1) * 128],
                            xn[:, ci, :],
                            start=(ci == 0), stop=(ci == CC - 1),
                        )
                    nc.vector.tensor_scalar_add(
                        qT_sb[:, co, qh * S:(qh + 1) * S], q_ps[:],
                        bq_sb[:, co:co + 1],
                    )

            # wp loads into wq's slot once wq is no longer needed
            wp_sb = const.tile([128, CC, C], F32R, tag="wqp", name="wp_sb")
            for ci in range(CC):
                nc.sync.dma_start(
                    out=wp_sb[:, ci, :],
                    in_=wp_d[ci * 128:(ci + 1) * 128, :].bitcast(F32R),
                )

            # ---- KV steps ----
            for t in range(NSTEP):
                xn = load_xn(2 + t, xkv_d, t * S)

                kT = kvpool.tile([128, CC, S], F32R, tag="kT", name="kT")
                for co in range(CC):
                    k_ps = psum_o.tile([128, S], F32, tag="proj", name="k_ps")
                    for ci in range(CC):
                        nc.tensor.matmul(
                            k_ps[:],
                            wk_sb[:, ci, co * 128:(co + 1) * 128],
                            xn[:, ci, :],
                            start=(ci == 0), stop=(ci == CC - 1),
                        )
                    nc.vector.tensor_scalar_add(
                        kT[:, co, :], k_ps[:], bk_sb[:, co:co + 1],
                    )

                v_sb = kvpool.tile([128, S // 128, C], F32R, tag="v", name="v_sb")
                for kp in range(S // 128):
                    v_ps = psum_o.tile([128, C], F32, tag="proj", name="v_ps")
                    for ci in range(CC):
                        nc.tensor.matmul(
                            v_ps[:],
                            xn[:, ci, kp * 128:(kp + 1) * 128],
                            wv_sb[:, ci, :],
                            start=(ci == 0), stop=(ci == CC - 1),
                        )
                    nc.vector.tensor_copy(v_sb[:, kp, :], v_ps[:])

                p_sb = ppool.tile([128, S // 128, Q], F32R, tag="p", name="p_sb")
                for kp in range(S // 128):
                    kvi = t * (S // 128) + kp
                    for qh in range(QH):
                        s_ps = psum_s.tile([128, S], F32, tag="s", name="s_ps")
                        for ci in range(CC):
                            nc.tensor.matmul(
                                s_ps[:],
                                kT[:, ci, kp * 128:(kp + 1) * 128],
                                qT_sb[:, ci, qh * S:(qh + 1) * S],
                                start=(ci == 0), stop=(ci == CC - 1),
                            )
                        nc.scalar.activation(
                            p_sb[:, kp, qh * S:(qh + 1) * S], s_ps[:],
                            Act.Exp, bias=kvb_sb[:, kvi:kvi + 1], scale=SCALE,
                        )

                # per-step softmax denominator partial, accumulated in SBUF
                for qh in range(QH):
                    dn_ps = psum_den.tile([1, S], F32, tag="small", name="dn_ps")
                    for kp in range(S // 128):
                        nc.tensor.matmul(
                            dn_ps[:], ones_sb[:],
                            p_sb[:, kp, qh * S:(qh + 1) * S],
                            start=(kp == 0), stop=(kp == S // 128 - 1),
                        )
                    nc.vector.tensor_add(
                        den_sb[:, qh * S:(qh + 1) * S],
                        den_sb[:, qh * S:(qh + 1) * S],
                        dn_ps[:],
                    )

                for co in range(CC):
                    for qh in range(QH):
                        o_ps = psum_o.tile([128, S], F32, tag="o", name="o_ps")
                        for kp in range(S // 128):
                            nc.tensor.matmul(
                                o_ps[:],
                                v_sb[:, kp, co * 128:(co + 1) * 128],
                                p_sb[:, kp, qh * S:(qh + 1) * S],
                                start=(kp == 0), stop=(kp == S // 128 - 1),
                            )
                        if t == 0:
                            nc.vector.tensor_copy(
                                o_sb[:, co, qh * S:(qh + 1) * S], o_ps[:]
                            )
                        else:
                            nc.vector.tensor_add(
                                o_sb[:, co, qh * S:(qh + 1) * S],
                                o_sb[:, co, qh * S:(qh + 1) * S],
                                o_ps[:],
                            )

            # ---- finalize: normalize, project, residual ----
            for qh in range(QH):
                rd = norm.tile([1, S], F32, tag="rn", name="rd")
                nc.vector.reciprocal(rd[:], den_sb[:, qh * S:(qh + 1) * S])
                nc.gpsimd.partition_broadcast(rdb[:, qh * S:(qh + 1) * S], rd[:])
            # o_n := o * (1/den), rounded to fp32r (reuses a p-pool slot)
            on_sb = ppool.tile([128, CC, Q], F32R, tag="p", name="on_sb")
            for ci in range(CC):
                nc.vector.tensor_mul(on_sb[:, ci, :], o_sb[:, ci, :], rdb[:])
            for qh in range(QH):
                xr = xload.tile([128, CC, S], F32, tag="xt", name="xr")
                for ci in range(CC):
                    nc.sync.dma_start(
                        out=xr[:, ci, :],
                        in_=xq_d[ci * 128:(ci + 1) * 128, qh * S:(qh + 1) * S],
                    )
                for co in range(CC):
                    pr_ps = psum_o.tile([128, S], F32, tag="proj", name="pr_ps")
                    for ci in range(CC):
                        nc.tensor.matmul(
                            pr_ps[:],
                            wp_sb[:, ci, co * 128:(co + 1) * 128],
                            on_sb[:, ci, qh * S:(qh + 1) * S],
                            start=(ci == 0), stop=(ci == CC - 1),
                        )
                    res = norm.tile([128, S], F32, tag="rnb", name="res")
                    nc.vector.scalar_tensor_tensor(
                        out=res[:],
                        in0=pr_ps[:],
                        scalar=bvp_sb[:, co:co + 1],
                        in1=xr[:, co, :],
                        op0=mybir.AluOpType.add,
                        op1=mybir.AluOpType.add,
                    )
                    nc.sync.dma_start(
                        out=out_d[co * 128:(co + 1) * 128, qh * S:(qh + 1) * S],
                        in_=res[:],
                    )

    nc.finalize()
    _cached["nc"] = nc
    return nc


def _prep_inputs(x, gamma, wq, bq, wk, bk, wv, bv, wp, bp):
    x = np.asarray(x, np.float32)
    X = np.ascontiguousarray(x[0].reshape(C, SEQ))
    g = (np.asarray(gamma, np.float32) * np.float32(np.sqrt(C))).astype(np.float32)
    wq = np.asarray(wq, np.float32)
    wk = np.asarray(wk, np.float32)
    wv = np.asarray(wv, np.float32)
    wp = np.asarray(wp, np.float32)
    bq = np.asarray(bq, np.float32)
    bk = np.asarray(bk, np.float32)
    bv = np.asarray(bv, np.float32)
    bp = np.asarray(bp, np.float32)
    wqT = np.ascontiguousarray((wq * g[None, :]).T)
    wkT = np.ascontiguousarray((wk * g[None, :]).T)
    wvT = np.ascontiguousarray((wv * g[None, :]).T)
    wpT = np.ascontiguousarray(wp.T)
    bvp = (bp + wp @ bv).astype(np.float32)

    common = {
        "xkv": X,
        "wqT": wqT, "wkT": wkT, "wvT": wvT, "wpT": wpT,
        "bq": np.ascontiguousarray(bq[:, None]),
        "bk": np.ascontiguousarray(bk[:, None]),
        "bvp": np.ascontiguousarray(bvp[:, None]),
    }
    in_maps = []
    for i in range(F):
        kvb = np.zeros((128, KVC), np.float32)
        for j in range(KVC):
            if j // (HW // 128) > i:
                kvb[:, j] = NEG
        m = dict(common)
        m["xq"] = np.ascontiguousarray(X[:, i * HW:(i + 1) * HW])
        m["kvb"] = kvb
        in_maps.append(m)
    return in_maps


def kernel(x, gamma, wq, bq, wk, bk, wv, bv, wp, bp, _trace=False):
    nc = _build()
    in_maps = _prep_inputs(x, gamma, wq, bq, wk, bk, wv, bv, wp, bp)
    kwargs = {}
    if _trace:
        kwargs = dict(trace=True, trace_cores=list(range(F)))
    r = run_bass_kernel_spmd(nc, in_maps, core_ids=list(range(F)), **kwargs)
    out = np.empty((1, C, F, 32, 32), np.float32)
    for i in range(F):
        out[0, :, i] = r.results[i]["out"].reshape(C, 32, 32)
    kernel._last_results = r
    return out


# revision 12
# speedup vs baseline: 1.1333x; 1.1333x over previous
"""Block-causal attention block (RMSnorm + QKV + frame-causal attention + proj)
on 8 TRN2 NeuronCores.

Sharding: sequence-parallel over the 8 frames — core i owns the 1024 queries of
frame i and processes KV blocks for frames 0..i (uniform SPMD program: all 16
half-blocks are processed on every core; future frames are killed by a
per-core additive bias of -1e30 before the exp, so they contribute exp() = 0
to both the numerator and denominator of the softmax).

Layouts are channel-first throughout ([C, seq] with C on partitions), which
makes every contraction a natural PE matmul with no transposes:
  k^T [C, kv]   = Wk' @ xn           (lhsT = wkT chunk, rhs = xn)
  v   [kv, C]   = xn^T @ Wv'^T       (lhsT = xn chunk,  rhs = wvT)
  S^T [kv, q]   = K @ Q^T            (lhsT = k^T chunk, rhs = q^T)
  O^T [C, q]    = V^T @ P^T          (lhsT = v chunk,   rhs = p^T)
  den [1, q]    = ones^T @ P^T       (lhsT = ones,      rhs = p^T)

Host-side folds: gamma*sqrt(C) into wq/wk/wv; bv through wp into the output
bias (softmax rows sum to 1); no max-subtraction in the softmax (scores here
are O(1); exp is safe and matches jax.nn.softmax exactly up to rounding).

All matmuls run in float32r (FP22) — full-rate on TRN2 with ~1e-4 accuracy.
"""

import sys

import numpy as np

sys.path.insert(0, "/opt/trn_rl_repo")

import concourse.bacc as bacc
import concourse.bass as bass  # noqa: F401
import concourse.tile as tile
from concourse import mybir
from concourse.bass_utils import run_bass_kernel_spmd

C = 512
CC = C // 128          # 4 channel chunks
F = 8                  # frames
HW = 1024              # tokens per frame
SEQ = F * HW           # 8192
S = 512                # kv columns processed per step
NSTEP = SEQ // S       # 16
Q = 1024               # queries per core (one frame)
QH = Q // S            # 2 query halves
KVC = SEQ // 128       # 64 kv 128-chunks
SCALE = 1.0 / float(np.sqrt(C))
NEG = -1.0e30

F32 = mybir.dt.float32
F32R = mybir.dt.float32r
Act = mybir.ActivationFunctionType

_cached = {}


def _build():
    if "nc" in _cached:
        return _cached["nc"]

    nc = bacc.Bacc()
    xq_d = nc.dram_tensor("xq", [C, Q], F32, kind="ExternalInput")
    xkv_d = nc.dram_tensor("xkv", [C, SEQ], F32, kind="ExternalInput")
    kvb_d = nc.dram_tensor("kvb", [128, KVC], F32, kind="ExternalInput")
    wq_d = nc.dram_tensor("wqT", [C, C], F32, kind="ExternalInput")
    wk_d = nc.dram_tensor("wkT", [C, C], F32, kind="ExternalInput")
    wv_d = nc.dram_tensor("wvT", [C, C], F32, kind="ExternalInput")
    wp_d = nc.dram_tensor("wpT", [C, C], F32, kind="ExternalInput")
    bq_d = nc.dram_tensor("bq", [C, 1], F32, kind="ExternalInput")
    bk_d = nc.dram_tensor("bk", [C, 1], F32, kind="ExternalInput")
    bvp_d = nc.dram_tensor("bvp", [C, 1], F32, kind="ExternalInput")
    out_d = nc.dram_tensor("out", [C, Q], F32, kind="ExternalOutput")

    with tile.TileContext(nc) as tc:
        with (
            tc.tile_pool(name="const", bufs=1) as const,
            tc.tile_pool(name="persist", bufs=1) as persist,
            tc.tile_pool(name="xload", bufs=3) as xload,
            tc.tile_pool(name="norm", bufs=2) as norm,
            tc.tile_pool(name="kv", bufs=2) as kvpool,
            tc.tile_pool(name="ppool", bufs=2) as ppool,
            tc.tile_pool(name="dram", bufs=1, space="DRAM") as drampool,
            tc.tile_pool(name="psum_s", bufs=2, space="PSUM") as psum_s,
            tc.tile_pool(name="psum_o", bufs=2, space="PSUM") as psum_o,
            tc.tile_pool(name="psum_den", bufs=2, space="PSUM") as psum_den,
        ):
            # ---- constants / weights (wq and wp share one slot: wp is only
            # needed after the last use of wq) ----
            wq_sb = const.tile([128, CC, C], F32R, tag="wqp", name="wq_sb")
            wk_sb = const.tile([128, CC, C], F32R, tag="wk", name="wk_sb")
            wv_sb = const.tile([128, CC, C], F32R, tag="wv", name="wv_sb")
            for w_sb, w_d in ((wq_sb, wq_d), (wk_sb, wk_d), (wv_sb, wv_d)):
                for ci in range(CC):
                    nc.sync.dma_start(
                        out=w_sb[:, ci, :],
                        in_=w_d[ci * 128:(ci + 1) * 128, :].bitcast(F32R),
                    )
            bq_sb = const.tile([128, CC], F32, tag="bq", name="bq_sb")
            bk_sb = const.tile([128, CC], F32, tag="bk", name="bk_sb")
            bvp_sb = const.tile([128, CC], F32, tag="bvp", name="bvp_sb")
            for b_sb, b_d in ((bq_sb, bq_d), (bk_sb, bk_d), (bvp_sb, bvp_d)):
                for ci in range(CC):
                    nc.sync.dma_start(
                        out=b_sb[:, ci:ci + 1],
                        in_=b_d[ci * 128:(ci + 1) * 128, :],
                    )
            kvb_sb = const.tile([128, KVC], F32, tag="kvb", name="kvb_sb")
            nc.sync.dma_start(out=kvb_sb[:], in_=kvb_d[:])
            ones_f = const.tile([128, 1], F32, tag="ones_f", name="ones_f")
            nc.vector.memset(ones_f[:], 1.0)
            ones_sb = const.tile([128, 1], F32R, tag="ones", name="ones_sb")
            nc.vector.tensor_copy(ones_sb[:], ones_f[:])
            # PE warmup: ~7us of back-to-back matmuls so the HAM clock gate
            # opens (4/8 -> 8/8) before the real matmul stream begins
            warm_f = const.tile([128, S], F32, tag="warm_f", name="warm_f")
            nc.vector.memset(warm_f[:], 0.0)
            warm_r = const.tile([128, S], F32R, tag="warm_r", name="warm_r")
            nc.vector.tensor_copy(warm_r[:], warm_f[:])
            warm_ps = psum_s.tile([1, S], F32, tag="s", name="warm_ps")
            for wi in range(20):
                nc.tensor.matmul(
                    warm_ps[:], ones_sb[:], warm_r[:],
                    start=(wi == 0), stop=(wi == 19),
                )

            wp_sb = const.tile([128, CC, C], F32R, tag="wqp", name="wp_sb")

            # ---- persistent q-side tiles ----
            qT_sb = persist.tile([128, CC, Q], F32R, tag="qT", name="qT_sb")
            o_sb = persist.tile([128, CC, Q], F32, tag="o", name="o_sb")
            rdb = persist.tile([128, Q], F32, tag="rdb", name="rdb")
            den_sb = persist.tile([1, Q], F32, tag="den_sb", name="den_sb")
            nc.vector.memset(den_sb[:], 0.0)

            # ---- single-load pipeline: unit u loads its x slab once,
            # computes column stats (ln of sum x^2 -> DRAM), and one unit
            # later the slab is normalized with the broadcast rnorm and fed
            # to the projections / attention step ----
            ln_dram = drampool.tile([18, S], F32, tag="ln_dram", name="ln_dram")
            UNITS = [(xq_d, qh * S) for qh in range(QH)]
            UNITS += [(xkv_d, t * S) for t in range(NSTEP)]
            LAG = 1
            xts = {}

            def stats_part(u):
                x_dram, col0 = UNITS[u]
                xt = xload.tile([128, CC, S], F32, tag="xt", name="xt")
                xts[u] = xt
                for ci in range(CC):
                    nc.sync.dma_start(
                        out=xt[:, ci, :],
                        in_=x_dram[ci * 128:(ci + 1) * 128, col0:col0 + S],
                    )
                ss_ps = psum_den.tile([1, S], F32, tag="small", name="ss_ps")
                for ci in range(CC):
                    xsq = norm.tile([128, S], F32R, tag="xsq", name="xsq")
                    nc.vector.tensor_mul(xsq[:], xt[:, ci, :], xt[:, ci, :])
                    nc.tensor.matmul(
                        ss_ps[:], ones_sb[:], xsq[:],
                        start=(ci == 0), stop=(ci == CC - 1),
                    )
                ln_t = norm.tile([1, S], F32, tag="ln_t", name="ln_t")
                nc.scalar.activation(ln_t[:], ss_ps[:], Act.Ln)
                nc.sync.dma_start(out=ln_dram[u:u + 1, :], in_=ln_t[:])

            def norm_slab(u):
                lnb = norm.tile([128, S], F32, tag="lnb", name="lnb")
                nc.sync.dma_start(
                    out=lnb[:], in_=ln_dram[u:u + 1, :].to_broadcast([128, S])
                )
                rnb = norm.tile([128, S], F32, tag="rnb", name="rnb")
                nc.scalar.activation(rnb[:], lnb[:], Act.Exp, scale=-0.5)
                xn = norm.tile([128, CC, S], F32R, tag="xn", name="xn")
                xt = xts.pop(u)
                for ci in range(CC):
                    nc.vector.tensor_mul(xn[:, ci, :], xt[:, ci, :], rnb[:])
                return xn

            def work_part(u):
                if u < QH:
                    qh = u
                    xn = norm_slab(u)
                    for co in range(CC):
                        q_ps = psum_o.tile([128, S], F32, tag="proj", name="q_ps")
                        for ci in range(CC):
                            nc.tensor.matmul(
                                q_ps[:],
                                wq_sb[:, ci, co * 128:(co + 1) * 128],
                                xn[:, ci, :],
                                start=(ci == 0), stop=(ci == CC - 1),
                            )
                        nc.vector.tensor_scalar_add(
                            qT_sb[:, co, qh * S:(qh + 1) * S], q_ps[:],
                            bq_sb[:, co:co + 1],
                        )
                    if u == QH - 1:
                        # wp loads into wq's slot once wq is no longer needed
                        for ci in range(CC):
                            nc.sync.dma_start(
                                out=wp_sb[:, ci, :],
                                in_=wp_d[ci * 128:(ci + 1) * 128, :].bitcast(F32R),
                            )
                    return
                t = u - QH
                xn = norm_slab(u)

                kT = kvpool.tile([128, CC, S], F32R, tag="kT", name="kT")
                for co in range(CC):
                    k_ps = psum_o.tile([128, S], F32, tag="proj", name="k_ps")
                    for ci in range(CC):
                        nc.tensor.matmul(
                            k_ps[:],
                            wk_sb[:, ci, co * 128:(co + 1) * 128],
                            xn[:, ci, :],
                            start=(ci == 0), stop=(ci == CC - 1),
                        )
                    nc.vector.tensor_scalar_add(
                        kT[:, co, :], k_ps[:], bk_sb[:, co:co + 1],
                    )

                v_sb = kvpool.tile([128, S // 128, C], F32R, tag="v", name="v_sb")
                for kp in range(S // 128):
                    v_ps = psum_o.tile([128, C], F32, tag="proj", name="v_ps")
                    for ci in range(CC):
                        nc.tensor.matmul(
                            v_ps[:],
                            xn[:, ci, kp * 128:(kp + 1) * 128],
                            wv_sb[:, ci, :],
                            start=(ci == 0), stop=(ci == CC - 1),
                        )
                    nc.vector.tensor_copy(v_sb[:, kp, :], v_ps[:])

                p_sb = ppool.tile([128, S // 128, Q], F32R, tag="p", name="p_sb")
                for kp in range(S // 128):
                    kvi = t * (S // 128) + kp
                    for qh in range(QH):
                        s_ps = psum_s.tile([128, S], F32, tag="s", name="s_ps")
                        for ci in range(CC):
                            nc.tensor.matmul(
                                s_ps[:],
                                kT[:, ci, kp * 128:(kp + 1) * 128],
                                qT_sb[:, ci, qh * S:(qh + 1) * S],
                                start=(ci == 0), stop=(ci == CC - 1),
                            )
                        nc.scalar.activation(
                            p_sb[:, kp, qh * S:(qh + 1) * S], s_ps[:],
                            Act.Exp, bias=kvb_sb[:, kvi:kvi + 1], scale=SCALE,
                        )

                for qh in range(QH):
                    dn_ps = psum_den.tile([1, S], F32, tag="small", name="dn_ps")
                    for kp in range(S // 128):
                        nc.tensor.matmul(
                            dn_ps[:], ones_sb[:],
                            p_sb[:, kp, qh * S:(qh + 1) * S],
                            start=(kp == 0), stop=(kp == S // 128 - 1),
                        )
                    nc.vector.tensor_add(
                        den_sb[:, qh * S:(qh + 1) * S],
                        den_sb[:, qh * S:(qh + 1) * S],
                        dn_ps[:],
                    )

                for co in range(CC):
                    for qh in range(QH):
                        o_ps = psum_o.tile([128, S], F32, tag="o", name="o_ps")
                        for kp in range(S // 128):
                            nc.tensor.matmul(
                                o_ps[:],
                                v_sb[:, kp, co * 128:(co + 1) * 128],
                                p_sb[:, kp, qh * S:(qh + 1) * S],
                                start=(kp == 0), stop=(kp == S // 128 - 1),
                            )
                        if t == 0:
                            nc.vector.tensor_copy(
                                o_sb[:, co, qh * S:(qh + 1) * S], o_ps[:]
                            )
                        else:
                            nc.vector.tensor_add(
                                o_sb[:, co, qh * S:(qh + 1) * S],
                                o_sb[:, co, qh * S:(qh + 1) * S],
                                o_ps[:],
                            )

            NU = len(UNITS)
            for i in range(NU + LAG):
                if i < NU:
                    stats_part(i)
                if i >= LAG:
                    work_part(i - LAG)

            # ---- finalize: normalize, project, residual ----
            for qh in range(QH):
                rd = norm.tile([1, S], F32, tag="rn", name="rd")
                nc.vector.reciprocal(rd[:], den_sb[:, qh * S:(qh + 1) * S])
                nc.gpsimd.partition_broadcast(rdb[:, qh * S:(qh + 1) * S], rd[:])
            # o_n := o * (1/den), rounded to fp32r (reuses a p-pool slot)
            on_sb = ppool.tile([128, CC, Q], F32R, tag="p", name="on_sb")
            for ci in range(CC):
                nc.vector.tensor_mul(on_sb[:, ci, :], o_sb[:, ci, :], rdb[:])
            for qh in range(QH):
                xr = xload.tile([128, CC, S], F32, tag="xt", name="xr")
                for ci in range(CC):
                    nc.sync.dma_start(
                        out=xr[:, ci, :],
                        in_=xq_d[ci * 128:(ci + 1) * 128, qh * S:(qh + 1) * S],
                    )
                for co in range(CC):
                    pr_ps = psum_o.tile([128, S], F32, tag="proj", name="pr_ps")
                    for ci in range(CC):
                        nc.tensor.matmul(
                            pr_ps[:],
                            wp_sb[:, ci, co * 128:(co + 1) * 128],
                            on_sb[:, ci, qh * S:(qh + 1) * S],
                            start=(ci == 0), stop=(ci == CC - 1),
                        )
                    res = norm.tile([128, S], F32, tag="rnb", name="res")
                    nc.vector.scalar_tensor_tensor(
                        out=res[:],
                        in0=pr_ps[:],
                        scalar=bvp_sb[:, co:co + 1],
                        in1=xr[:, co, :],
                        op0=mybir.AluOpType.add,
                        op1=mybir.AluOpType.add,
                    )
                    nc.sync.dma_start(
                        out=out_d[co * 128:(co + 1) * 128, qh * S:(qh + 1) * S],
                        in_=res[:],
                    )

    nc.finalize()
    _cached["nc"] = nc
    return nc


def _prep_inputs(x, gamma, wq, bq, wk, bk, wv, bv, wp, bp):
    x = np.asarray(x, np.float32)
    X = np.ascontiguousarray(x[0].reshape(C, SEQ))
    g = (np.asarray(gamma, np.float32) * np.float32(np.sqrt(C))).astype(np.float32)
    wq = np.asarray(wq, np.float32)
    wk = np.asarray(wk, np.float32)
    wv = np.asarray(wv, np.float32)
    wp = np.asarray(wp, np.float32)
    bq = np.asarray(bq, np.float32)
    bk = np.asarray(bk, np.float32)
    bv = np.asarray(bv, np.float32)
    bp = np.asarray(bp, np.float32)
    wqT = np.ascontiguousarray((wq * g[None, :]).T)
    wkT = np.ascontiguousarray((wk * g[None, :]).T)
    wvT = np.ascontiguousarray((wv * g[None, :]).T)
    wpT = np.ascontiguousarray(wp.T)
    bvp = (bp + wp @ bv).astype(np.float32)

    common = {
        "xkv": X,
        "wqT": wqT, "wkT": wkT, "wvT": wvT, "wpT": wpT,
        "bq": np.ascontiguousarray(bq[:, None]),
        "bk": np.ascontiguousarray(bk[:, None]),
        "bvp": np.ascontiguousarray(bvp[:, None]),
    }
    in_maps = []
    for i in range(F):
        kvb = np.zeros((128, KVC), np.float32)
        for j in range(KVC):
            if j // (HW // 128) > i:
                kvb[:, j] = NEG
        m = dict(common)
        m["xq"] = np.ascontiguousarray(X[:, i * HW:(i + 1) * HW])
        m["kvb"] = kvb
        in_maps.append(m)
    return in_maps


def kernel(x, gamma, wq, bq, wk, bk, wv, bv, wp, bp, _trace=False):
    nc = _build()
    in_maps = _prep_inputs(x, gamma, wq, bq, wk, bk, wv, bv, wp, bp)
    kwargs = {}
    if _trace:
        kwargs = dict(trace=True, trace_cores=list(range(F)))
    r = run_bass_kernel_spmd(nc, in_maps, core_ids=list(range(F)), **kwargs)
    out = np.empty((1, C, F, 32, 32), np.float32)
    for i in range(F):
        out[0, :, i] = r.results[i]["out"].reshape(C, 32, 32)
    kernel._last_results = r
    return out


# revision 13
# speedup vs baseline: 1.3201x; 1.1648x over previous
"""Block-causal attention block (RMSnorm + QKV + frame-causal attention + proj)
on 8 TRN2 NeuronCores.

Sharding: sequence-parallel over the 8 frames — core i owns the 1024 queries of
frame i and processes KV blocks for frames 0..i (uniform SPMD program: all 16
half-blocks are processed on every core; future frames are killed by a
per-core additive bias of -1e30 before the exp, so they contribute exp() = 0
to both the numerator and denominator of the softmax).

Layouts are channel-first throughout ([C, seq] with C on partitions), which
makes every contraction a natural PE matmul with no transposes:
  k^T [C, kv]   = Wk' @ xn           (lhsT = wkT chunk, rhs = xn)
  v   [kv, C]   = xn^T @ Wv'^T       (lhsT = xn chunk,  rhs = wvT)
  S^T [kv, q]   = K @ Q^T            (lhsT = k^T chunk, rhs = q^T)
  O^T [C, q]    = V^T @ P^T          (lhsT = v chunk,   rhs = p^T)
  den [1, q]    = ones^T @ P^T       (lhsT = ones,      rhs = p^T)

Host-side folds: gamma*sqrt(C) into wq/wk/wv; bv through wp into the output
bias (softmax rows sum to 1); no max-subtraction in the softmax (scores here
are O(1); exp is safe and matches jax.nn.softmax exactly up to rounding).

All matmuls run in float32r (FP22) — full-rate on TRN2 with ~1e-4 accuracy.
"""

import sys

import numpy as np

sys.path.insert(0, "/opt/trn_rl_repo")

import concourse.bacc as bacc
import concourse.bass as bass  # noqa: F401
import concourse.tile as tile
from concourse import mybir
from concourse.bass_utils import run_bass_kernel_spmd

C = 512
CC = C // 128          # 4 channel chunks
F = 8                  # frames
HW = 1024              # tokens per frame
SEQ = F * HW           # 8192
S = 512                # kv columns processed per step
KSTEPS = 18            # folded kv half-steps per core (perfectly balanced)
SEQF = KSTEPS * S      # folded kv stream width
Q = 1024               # queries per core (two half-frames: one early, one late)
QH = Q // S            # 2 query halves
SCALE = 1.0 / float(np.sqrt(C))
NEG = -1.0e30

F32 = mybir.dt.float32
F32R = mybir.dt.float32r
Act = mybir.ActivationFunctionType

_cached = {}


def _build():
    if "nc" in _cached:
        return _cached["nc"]

    nc = bacc.Bacc()
    xq_d = nc.dram_tensor("xq", [C, Q], F32, kind="ExternalInput")
    xkv_d = nc.dram_tensor("xkv", [C, SEQF], F32, kind="ExternalInput")
    qoff_d = nc.dram_tensor("qoff", [1, KSTEPS], mybir.dt.int32, kind="ExternalInput")
    wq_d = nc.dram_tensor("wqT", [C, C], F32, kind="ExternalInput")
    wk_d = nc.dram_tensor("wkT", [C, C], F32, kind="ExternalInput")
    wv_d = nc.dram_tensor("wvT", [C, C], F32, kind="ExternalInput")
    wp_d = nc.dram_tensor("wpT", [C, C], F32, kind="ExternalInput")
    bq_d = nc.dram_tensor("bq", [C, 1], F32, kind="ExternalInput")
    bk_d = nc.dram_tensor("bk", [C, 1], F32, kind="ExternalInput")
    bvp_d = nc.dram_tensor("bvp", [C, 1], F32, kind="ExternalInput")
    out_d = nc.dram_tensor("out", [C, Q], F32, kind="ExternalOutput")

    with tile.TileContext(nc) as tc:
        with (
            tc.tile_pool(name="const", bufs=1) as const,
            tc.tile_pool(name="persist", bufs=1) as persist,
            tc.tile_pool(name="xload", bufs=3) as xload,
            tc.tile_pool(name="norm", bufs=2) as norm,
            tc.tile_pool(name="kv", bufs=2) as kvpool,
            tc.tile_pool(name="ppool", bufs=2) as ppool,
            tc.tile_pool(name="dram", bufs=1, space="DRAM") as drampool,
            tc.tile_pool(name="psum_s", bufs=2, space="PSUM") as psum_s,
            tc.tile_pool(name="psum_o", bufs=2, space="PSUM") as psum_o,
            tc.tile_pool(name="psum_den", bufs=2, space="PSUM") as psum_den,
        ):
            # ---- constants / weights (wq and wp share one slot: wp is only
            # needed after the last use of wq) ----
            wq_sb = const.tile([128, CC, C], F32R, tag="wqp", name="wq_sb")
            wk_sb = const.tile([128, CC, C], F32R, tag="wk", name="wk_sb")
            wv_sb = const.tile([128, CC, C], F32R, tag="wv", name="wv_sb")
            for w_sb, w_d in ((wq_sb, wq_d), (wk_sb, wk_d), (wv_sb, wv_d)):
                for ci in range(CC):
                    nc.sync.dma_start(
                        out=w_sb[:, ci, :],
                        in_=w_d[ci * 128:(ci + 1) * 128, :].bitcast(F32R),
                    )
            bq_sb = const.tile([128, CC], F32, tag="bq", name="bq_sb")
            bk_sb = const.tile([128, CC], F32, tag="bk", name="bk_sb")
            bvp_sb = const.tile([128, CC], F32, tag="bvp", name="bvp_sb")
            for b_sb, b_d in ((bq_sb, bq_d), (bk_sb, bk_d), (bvp_sb, bvp_d)):
                for ci in range(CC):
                    nc.sync.dma_start(
                        out=b_sb[:, ci:ci + 1],
                        in_=b_d[ci * 128:(ci + 1) * 128, :],
                    )
            qoff_sb = const.tile([1, KSTEPS], mybir.dt.int32, tag="qoff", name="qoff_sb")
            nc.sync.dma_start(out=qoff_sb[:], in_=qoff_d[:])
            ones_f = const.tile([128, 1], F32, tag="ones_f", name="ones_f")
            nc.vector.memset(ones_f[:], 1.0)
            ones_sb = const.tile([128, 1], F32R, tag="ones", name="ones_sb")
            nc.vector.tensor_copy(ones_sb[:], ones_f[:])
            # PE warmup: ~7us of back-to-back matmuls so the HAM clock gate
            # opens (4/8 -> 8/8) before the real matmul stream begins
            warm_f = const.tile([128, S], F32, tag="warm_f", name="warm_f")
            nc.vector.memset(warm_f[:], 0.0)
            warm_r = const.tile([128, S], F32R, tag="warm_r", name="warm_r")
            nc.vector.tensor_copy(warm_r[:], warm_f[:])
            warm_ps = psum_s.tile([1, S], F32, tag="s", name="warm_ps")
            for wi in range(20):
                nc.tensor.matmul(
                    warm_ps[:], ones_sb[:], warm_r[:],
                    start=(wi == 0), stop=(wi == 19),
                )

            wp_sb = const.tile([128, CC, C], F32R, tag="wqp", name="wp_sb")

            # ---- persistent q-side tiles ----
            qT_sb = persist.tile([128, CC, Q], F32R, tag="qT", name="qT_sb")
            o_sb = persist.tile([128, CC, Q], F32, tag="o", name="o_sb")
            nc.vector.memset(o_sb[:], 0.0)
            rdb = persist.tile([128, Q], F32, tag="rdb", name="rdb")
            den_sb = persist.tile([1, Q], F32, tag="den_sb", name="den_sb")
            nc.vector.memset(den_sb[:], 0.0)

            # ---- single-load pipeline: unit u loads its x slab once,
            # computes column stats (ln of sum x^2 -> DRAM), and one unit
            # later the slab is normalized with the broadcast rnorm and fed
            # to the projections / attention step ----
            ln_dram = drampool.tile([2 + KSTEPS, S], F32, tag="ln_dram", name="ln_dram")
            UNITS = [(xq_d, qh * S) for qh in range(QH)]
            UNITS += [(xkv_d, t * S) for t in range(KSTEPS)]
            LAG = 1
            xts = {}

            def stats_part(u):
                x_dram, col0 = UNITS[u]
                xt = xload.tile([128, CC, S], F32, tag="xt", name="xt")
                xts[u] = xt
                for ci in range(CC):
                    nc.sync.dma_start(
                        out=xt[:, ci, :],
                        in_=x_dram[ci * 128:(ci + 1) * 128, col0:col0 + S],
                    )
                ss_ps = psum_den.tile([1, S], F32, tag="small", name="ss_ps")
                for ci in range(CC):
                    xsq = norm.tile([128, S], F32R, tag="xsq", name="xsq")
                    nc.vector.tensor_mul(xsq[:], xt[:, ci, :], xt[:, ci, :])
                    nc.tensor.matmul(
                        ss_ps[:], ones_sb[:], xsq[:],
                        start=(ci == 0), stop=(ci == CC - 1),
                    )
                ln_t = norm.tile([1, S], F32, tag="ln_t", name="ln_t")
                nc.scalar.activation(ln_t[:], ss_ps[:], Act.Ln)
                nc.sync.dma_start(out=ln_dram[u:u + 1, :], in_=ln_t[:])

            def norm_slab(u):
                lnb = norm.tile([128, S], F32, tag="lnb", name="lnb")
                nc.sync.dma_start(
                    out=lnb[:], in_=ln_dram[u:u + 1, :].to_broadcast([128, S])
                )
                rnb = norm.tile([128, S], F32, tag="rnb", name="rnb")
                nc.scalar.activation(rnb[:], lnb[:], Act.Exp, scale=-0.5)
                xn = norm.tile([128, CC, S], F32R, tag="xn", name="xn")
                xt = xts.pop(u)
                for ci in range(CC):
                    nc.vector.tensor_mul(xn[:, ci, :], xt[:, ci, :], rnb[:])
                return xn

            def work_part(u):
                if u < QH:
                    qh = u
                    xn = norm_slab(u)
                    for co in range(CC):
                        q_ps = psum_o.tile([128, S], F32, tag="proj", name="q_ps")
                        for ci in range(CC):
                            nc.tensor.matmul(
                                q_ps[:],
                                wq_sb[:, ci, co * 128:(co + 1) * 128],
                                xn[:, ci, :],
                                start=(ci == 0), stop=(ci == CC - 1),
                            )
                        nc.vector.tensor_scalar_add(
                            qT_sb[:, co, qh * S:(qh + 1) * S], q_ps[:],
                            bq_sb[:, co:co + 1],
                        )
                    if u == QH - 1:
                        # wp loads into wq's slot once wq is no longer needed
                        for ci in range(CC):
                            nc.sync.dma_start(
                                out=wp_sb[:, ci, :],
                                in_=wp_d[ci * 128:(ci + 1) * 128, :].bitcast(F32R),
                            )
                    return
                t = u - QH
                xn = norm_slab(u)
                off = nc.values_load(
                    qoff_sb[0:1, t:t + 1],
                    engines=[mybir.EngineType.PE, mybir.EngineType.DVE],
                    min_val=0, max_val=S,
                    skip_runtime_bounds_check=True,
                )

                kT = kvpool.tile([128, CC, S], F32R, tag="kT", name="kT")
                for co in range(CC):
                    k_ps = psum_o.tile([128, S], F32, tag="proj", name="k_ps")
                    for ci in range(CC):
                        nc.tensor.matmul(
                            k_ps[:],
                            wk_sb[:, ci, co * 128:(co + 1) * 128],
                            xn[:, ci, :],
                            start=(ci == 0), stop=(ci == CC - 1),
                        )
                    nc.vector.tensor_scalar_add(
                        kT[:, co, :], k_ps[:], bk_sb[:, co:co + 1],
                    )

                v_sb = kvpool.tile([128, S // 128, C], F32R, tag="v", name="v_sb")
                for kp in range(S // 128):
                    v_ps = psum_o.tile([128, C], F32, tag="proj", name="v_ps")
                    for ci in range(CC):
                        nc.tensor.matmul(
                            v_ps[:],
                            xn[:, ci, kp * 128:(kp + 1) * 128],
                            wv_sb[:, ci, :],
                            start=(ci == 0), stop=(ci == CC - 1),
                        )
                    nc.vector.tensor_copy(v_sb[:, kp, :], v_ps[:])

                p_sb = ppool.tile([128, S // 128, S], F32R, tag="p", name="p_sb")
                for kp in range(S // 128):
                    s_ps = psum_s.tile([128, S], F32, tag="s", name="s_ps")
                    for ci in range(CC):
                        nc.tensor.matmul(
                            s_ps[:],
                            kT[:, ci, kp * 128:(kp + 1) * 128],
                            qT_sb[:, ci, bass.ds(off, S)],
                            start=(ci == 0), stop=(ci == CC - 1),
                        )
                    nc.scalar.activation(
                        p_sb[:, kp, :], s_ps[:], Act.Exp, bias=0.0, scale=SCALE,
                    )

                dn_ps = psum_den.tile([1, S], F32, tag="small", name="dn_ps")
                for kp in range(S // 128):
                    nc.tensor.matmul(
                        dn_ps[:], ones_sb[:], p_sb[:, kp, :],
                        start=(kp == 0), stop=(kp == S // 128 - 1),
                    )
                nc.vector.tensor_add(
                    den_sb[:, bass.ds(off, S)],
                    den_sb[:, bass.ds(off, S)],
                    dn_ps[:],
                )

                for co in range(CC):
                    o_ps = psum_o.tile([128, S], F32, tag="o", name="o_ps")
                    for kp in range(S // 128):
                        nc.tensor.matmul(
                            o_ps[:],
                            v_sb[:, kp, co * 128:(co + 1) * 128],
                            p_sb[:, kp, :],
                            start=(kp == 0), stop=(kp == S // 128 - 1),
                        )
                    nc.vector.tensor_add(
                        o_sb[:, co, bass.ds(off, S)],
                        o_sb[:, co, bass.ds(off, S)],
                        o_ps[:],
                    )

            NU = len(UNITS)
            for i in range(NU + LAG):
                if i < NU:
                    stats_part(i)
                if i >= LAG:
                    work_part(i - LAG)

            # ---- finalize: normalize, project, residual ----
            for qh in range(QH):
                rd = norm.tile([1, S], F32, tag="rn", name="rd")
                nc.vector.reciprocal(rd[:], den_sb[:, qh * S:(qh + 1) * S])
                nc.gpsimd.partition_broadcast(rdb[:, qh * S:(qh + 1) * S], rd[:])
            # o_n := o * (1/den), rounded to fp32r (reuses a p-pool slot)
            on_sb = ppool.tile([128, CC, Q], F32R, tag="on", name="on_sb", bufs=1)
            for ci in range(CC):
                nc.vector.tensor_mul(on_sb[:, ci, :], o_sb[:, ci, :], rdb[:])
            for qh in range(QH):
                xr = xload.tile([128, CC, S], F32, tag="xt", name="xr")
                for ci in range(CC):
                    nc.sync.dma_start(
                        out=xr[:, ci, :],
                        in_=xq_d[ci * 128:(ci + 1) * 128, qh * S:(qh + 1) * S],
                    )
                for co in range(CC):
                    pr_ps = psum_o.tile([128, S], F32, tag="proj", name="pr_ps")
                    for ci in range(CC):
                        nc.tensor.matmul(
                            pr_ps[:],
                            wp_sb[:, ci, co * 128:(co + 1) * 128],
                            on_sb[:, ci, qh * S:(qh + 1) * S],
                            start=(ci == 0), stop=(ci == CC - 1),
                        )
                    res = norm.tile([128, S], F32, tag="rnb", name="res")
                    nc.vector.scalar_tensor_tensor(
                        out=res[:],
                        in0=pr_ps[:],
                        scalar=bvp_sb[:, co:co + 1],
                        in1=xr[:, co, :],
                        op0=mybir.AluOpType.add,
                        op1=mybir.AluOpType.add,
                    )
                    nc.sync.dma_start(
                        out=out_d[co * 128:(co + 1) * 128, qh * S:(qh + 1) * S],
                        in_=res[:],
                    )

    nc.finalize()
    _cached["nc"] = nc
    return nc


def _prep_inputs(x, gamma, wq, bq, wk, bk, wv, bv, wp, bp):
    x = np.asarray(x, np.float32)
    X = np.ascontiguousarray(x[0].reshape(C, SEQ))
    g = (np.asarray(gamma, np.float32) * np.float32(np.sqrt(C))).astype(np.float32)
    wq = np.asarray(wq, np.float32)
    wk = np.asarray(wk, np.float32)
    wv = np.asarray(wv, np.float32)
    wp = np.asarray(wp, np.float32)
    bq = np.asarray(bq, np.float32)
    bk = np.asarray(bk, np.float32)
    bv = np.asarray(bv, np.float32)
    bp = np.asarray(bp, np.float32)
    wqT = np.ascontiguousarray((wq * g[None, :]).T)
    wkT = np.ascontiguousarray((wk * g[None, :]).T)
    wvT = np.ascontiguousarray((wv * g[None, :]).T)
    wpT = np.ascontiguousarray(wp.T)
    bvp = (bp + wp @ bv).astype(np.float32)

    common = {
        "wqT": wqT, "wkT": wkT, "wvT": wvT, "wpT": wpT,
        "bq": np.ascontiguousarray(bq[:, None]),
        "bk": np.ascontiguousarray(bk[:, None]),
        "bvp": np.ascontiguousarray(bvp[:, None]),
    }
    in_maps = []
    for j in range(F):
        p, half = j // 2, j % 2
        fa, fb = p, F - 1 - p
        qa = X[:, fa * HW + half * S: fa * HW + half * S + S]
        qb = X[:, fb * HW + half * S: fb * HW + half * S + S]
        na, nb = 2 * (fa + 1), 2 * (fb + 1)
        assert na + nb == KSTEPS
        cols = []
        for hf in range(na):
            cols.append(X[:, hf * S:(hf + 1) * S])
        for hf in range(nb):
            cols.append(X[:, hf * S:(hf + 1) * S])
        m = dict(common)
        m["xq"] = np.ascontiguousarray(np.concatenate([qa, qb], axis=1))
        m["xkv"] = np.ascontiguousarray(np.concatenate(cols, axis=1))
        m["qoff"] = np.asarray(
            [[0] * na + [S] * nb], np.int32
        )
        in_maps.append(m)
    return in_maps


def kernel(x, gamma, wq, bq, wk, bk, wv, bv, wp, bp, _trace=False):
    nc = _build()
    in_maps = _prep_inputs(x, gamma, wq, bq, wk, bk, wv, bv, wp, bp)
    kwargs = {}
    if _trace:
        kwargs = dict(trace=True, trace_cores=list(range(F)))
    r = run_bass_kernel_spmd(nc, in_maps, core_ids=list(range(F)), **kwargs)
    out = np.empty((1, C, F, HW), np.float32)
    for j in range(F):
        p, half = j // 2, j % 2
        fa, fb = p, F - 1 - p
        res = r.results[j]["out"]
        out[0, :, fa, half * S:half * S + S] = res[:, 0:S]
        out[0, :, fb, half * S:half * S + S] = res[:, S:Q]
    out = out.reshape(1, C, F, 32, 32)
    kernel._last_results = r
    return out
